# revision 1
# baseline (speedup 1.0000x reference)
"""DLRM forward (embedding gather + tiny MLPs) as a Bass/Tile kernel on 8 trn2 cores.

Sharding: data-parallel over the batch. Each of the 8 cores gets B/8 = 2048 rows
of dense_x / sparse_x plus a full replica of the (read-only) embedding tables,
computes its batch shard end-to-end on device, and returns [1, 2048] sigmoid
outputs. The host only slices inputs and concatenates outputs.

Per-core pipeline:
  - combined index = sparse_idx + f*CARD (iota + DVE add), tables viewed flat
    [26*100000, 64] so one indirect DMA per 128-row batch tile gathers all 26
    embedding rows per sample: [128, 26] idxs -> [128, 1664] f32.
  - PE transposes 128x128 feature chunks (features -> partitions), DVE/ACT
    copy-casts PSUM -> SBUF bf16, PE matmuls against bf16 tw1 chunks accumulate
    the top-MLP hidden layer [16, 512] per 512-sample group.
  - bottom MLP runs transposed ([13,512] -> [8,512] -> [64,512]) and feeds the
    last accumulation chunk. ACT applies biases/relu/sigmoid.
"""

import numpy as np

import concourse.bass as bass
import concourse.mybir as mybir
import concourse.tile as tile
from concourse import bacc
from concourse.masks import make_identity

P = 128

# Problem constants (hardcoded per harness contract).
N_CORES = 8
B = 16384
F = 26
D = 64
DENSE = 13
CARD = 100000
H_BOT = 8
H_TOP = 16

f32 = mybir.dt.float32
i32 = mybir.dt.int32
bf16 = mybir.dt.bfloat16
fp16 = mybir.dt.float16


def build_kernel(
    b_loc=B // N_CORES,
    card=CARD,
    n_f=F,
    d=D,
    n_dense=DENSE,
    h_bot=H_BOT,
    h_top=H_TOP,
    compute_dt=fp16,
    table_dt=fp16,
):
    v = n_f * card
    k_emb = n_f * d
    assert k_emb % P == 0
    kc_n = k_emb // P  # feature chunks of 128
    group = min(512, b_loc)  # batch columns per matmul group
    tpg = group // P  # 128-row tiles per group
    n_g = b_loc // group
    assert b_loc % group == 0 and group % P == 0

    # Bacc (not raw Bass): its compile() pipeline legalizes semaphore waits
    # (TRN2 allows one wait per instruction) via generate_event_semaphores.
    nc = bacc.Bacc("TRN2", target_bir_lowering=False)
    dense_d = nc.dram_tensor("dense_x", [b_loc, n_dense], f32, kind="ExternalInput")
    sparse_d = nc.dram_tensor("sparse_x", [b_loc, n_f], i32, kind="ExternalInput")
    tables_d = nc.dram_tensor("tables", [v, d], table_dt, kind="ExternalInput")
    w1_d = nc.dram_tensor("w1", [n_dense, h_bot], f32, kind="ExternalInput")
    b1_d = nc.dram_tensor("b1", [h_bot], f32, kind="ExternalInput")
    w2_d = nc.dram_tensor("w2", [h_bot, d], f32, kind="ExternalInput")
    b2_d = nc.dram_tensor("b2", [d], f32, kind="ExternalInput")
    tw1_d = nc.dram_tensor("tw1", [k_emb + d, h_top], f32, kind="ExternalInput")
    tb1_d = nc.dram_tensor("tb1", [h_top], f32, kind="ExternalInput")
    tw2_d = nc.dram_tensor("tw2", [h_top, 1], f32, kind="ExternalInput")
    tb2_d = nc.dram_tensor("tb2", [1], f32, kind="ExternalInput")
    y_d = nc.dram_tensor("y", [1, b_loc], f32, kind="ExternalOutput")

    n_t = b_loc // P

    with tile.TileContext(nc) as tc:
        with (
            tc.tile_pool(name="const", bufs=1) as cpool,
            tc.tile_pool(name="emb", bufs=6) as embp,
            tc.tile_pool(name="embT", bufs=4) as embtp,
            tc.tile_pool(name="dx", bufs=3) as dxp,
            tc.tile_pool(name="small", bufs=2) as smallp,
            tc.tile_pool(name="ptr", bufs=3, space="PSUM") as ptrp,
            tc.tile_pool(name="po1", bufs=2, space="PSUM") as po1p,
            tc.tile_pool(name="psmall", bufs=2, space="PSUM") as psmallp,
        ):
            # ---- constants / weights ----
            ident = cpool.tile([P, P], f32)
            make_identity(nc, ident[:])
            if table_dt == f32:
                ident_t = ident
            else:
                ident_t = cpool.tile([P, P], table_dt)
                make_identity(nc, ident_t[:])

            # per-sample table base offsets: fofs[p, t, f] = f * card
            # (iota pattern steps are int16-limited, so generate f then scale;
            # keep the whole chain on gpsimd — walrus allows only one sync
            # wait on TensorTensor-class instructions, and a single-engine
            # chain needs just the one DMA wait)
            fidx = cpool.tile([P, n_t * n_f], i32)
            nc.gpsimd.iota(
                fidx[:], pattern=[[0, n_t], [1, n_f]], base=0, channel_multiplier=0
            )
            fofs = cpool.tile([P, n_t * n_f], i32)
            nc.gpsimd.tensor_scalar_mul(fofs[:], fidx[:], card)
            idx_raw = cpool.tile([P, n_t * n_f], i32)
            nc.sync.dma_start(
                out=idx_raw[:].rearrange("p (t f) -> p t f", t=n_t),
                in_=sparse_d[:, :].rearrange("(t p) f -> p t f", p=P),
            )
            # TensorTensor-class instructions have a single ISA wait slot, so
            # stage through a same-engine copy: the copy absorbs the DMA wait
            # into Pool's vector clock, the add then only self-waits on Pool.
            comb = cpool.tile([P, n_t * n_f], i32)
            nc.gpsimd.tensor_copy(out=comb[:], in_=idx_raw[:])
            nc.gpsimd.tensor_tensor(
                out=comb[:], in0=comb[:], in1=fofs[:], op=mybir.AluOpType.add
            )

            tw1_f = cpool.tile([P, kc_n * h_top], f32)
            nc.sync.dma_start(
                out=tw1_f[:].rearrange("p (c m) -> p c m", c=kc_n),
                in_=tw1_d[0:k_emb, :].rearrange("(c p) m -> p c m", p=P),
            )
            tw1_c = cpool.tile([P, kc_n * h_top], compute_dt)
            nc.vector.tensor_copy(out=tw1_c[:], in_=tw1_f[:])

            tw1d_f = cpool.tile([d, h_top], f32)
            nc.sync.dma_start(out=tw1d_f[:], in_=tw1_d[k_emb : k_emb + d, :])
            tw1d_c = cpool.tile([d, h_top], compute_dt)
            nc.vector.tensor_copy(out=tw1d_c[:], in_=tw1d_f[:])

            tw2_f = cpool.tile([h_top, 1], f32)
            nc.sync.dma_start(out=tw2_f[:], in_=tw2_d[:, :])
            tw2_c = cpool.tile([h_top, 1], compute_dt)
            nc.vector.tensor_copy(out=tw2_c[:], in_=tw2_f[:])

            w1_sb = cpool.tile([n_dense, h_bot], f32)
            nc.sync.dma_start(out=w1_sb[:], in_=w1_d[:, :])
            w2_sb = cpool.tile([h_bot, d], f32)
            nc.sync.dma_start(out=w2_sb[:], in_=w2_d[:, :])
            b1_sb = cpool.tile([h_bot, 1], f32)
            nc.sync.dma_start(out=b1_sb[:], in_=b1_d[:, None])
            b2_sb = cpool.tile([d, 1], f32)
            nc.sync.dma_start(out=b2_sb[:], in_=b2_d[:, None])
            tb1_sb = cpool.tile([h_top, 1], f32)
            nc.sync.dma_start(out=tb1_sb[:], in_=tb1_d[:, None])
            tb2_sb = cpool.tile([1, 1], f32)
            nc.sync.dma_start(out=tb2_sb[:], in_=tb2_d[:, None])

            y_row = cpool.tile([1, b_loc], f32)

            for g in range(n_g):
                # ---- embedding gathers: one indirect DMA per 128-sample tile ----
                emb_tiles = []
                for j in range(tpg):
                    t = g * tpg + j
                    et = embp.tile([P, k_emb], table_dt, tag="emb")
                    nc.gpsimd.indirect_dma_start(
                        out=et[:],
                        out_offset=None,
                        in_=tables_d[:, :],
                        in_offset=bass.IndirectOffsetOnAxis(
                            ap=comb[:, t * n_f : (t + 1) * n_f], axis=0
                        ),
                    )
                    emb_tiles.append(et)

                # ---- bottom MLP (transposed layout) ----
                pdx = psmallp.tile([n_dense, group], f32, tag="psmall")
                for j in range(tpg):
                    t = g * tpg + j
                    dx_t = dxp.tile([P, n_dense], f32, tag="dx")
                    nc.sync.dma_start(out=dx_t[:], in_=dense_d[bass.ts(t, P), :])
                    nc.tensor.transpose(
                        out=pdx[:, bass.ts(j, P)], in_=dx_t[:], identity=ident[:]
                    )
                dxt = smallp.tile([n_dense, group], f32, tag="dxt")
                nc.vector.tensor_copy(out=dxt[:], in_=pdx[:])
                ph = psmallp.tile([h_bot, group], f32, tag="psmall")
                nc.tensor.matmul(out=ph[:], lhsT=w1_sb[:], rhs=dxt[:], start=True, stop=True)
                h_s = smallp.tile([h_bot, group], f32, tag="h")
                nc.scalar.activation(
                    out=h_s[:],
                    in_=ph[:],
                    func=mybir.ActivationFunctionType.Relu,
                    bias=b1_sb[:],
                )
                pd = psmallp.tile([d, group], f32, tag="psmall")
                nc.tensor.matmul(out=pd[:], lhsT=w2_sb[:], rhs=h_s[:], start=True, stop=True)
                dt_sb = smallp.tile([d, group], compute_dt, tag="dt")
                nc.scalar.activation(
                    out=dt_sb[:],
                    in_=pd[:],
                    func=mybir.ActivationFunctionType.Identity,
                    bias=b2_sb[:],
                )

                # ---- top MLP layer 1: transpose feature chunks, accumulate ----
                po1 = po1p.tile([h_top, group], f32, tag="po1")
                for kc in range(kc_n):
                    ptr = ptrp.tile([P, group], table_dt, tag="ptr")
                    for j in range(tpg):
                        nc.tensor.transpose(
                            out=ptr[:, bass.ts(j, P)],
                            in_=emb_tiles[j][:, bass.ts(kc, P)],
                            identity=ident_t[:],
                        )
                    embt = embtp.tile([P, group], compute_dt, tag="embT")
                    if kc % 2 == 0:
                        nc.vector.tensor_copy(out=embt[:], in_=ptr[:])
                    else:
                        nc.scalar.activation(
                            out=embt[:],
                            in_=ptr[:],
                            func=mybir.ActivationFunctionType.Copy,
                        )
                    nc.tensor.matmul(
                        out=po1[:],
                        lhsT=tw1_c[:, bass.ts(kc, h_top)],
                        rhs=embt[:],
                        start=(kc == 0),
                        stop=False,
                    )
                nc.tensor.matmul(
                    out=po1[:], lhsT=tw1d_c[:], rhs=dt_sb[:], start=False, stop=True
                )

                o1 = smallp.tile([h_top, group], compute_dt, tag="o1")
                nc.scalar.activation(
                    out=o1[:],
                    in_=po1[:],
                    func=mybir.ActivationFunctionType.Relu,
                    bias=tb1_sb[:],
                )
                plg = psmallp.tile([1, group], f32, tag="psmall")
                nc.tensor.matmul(out=plg[:], lhsT=tw2_c[:], rhs=o1[:], start=True, stop=True)
                nc.scalar.activation(
                    out=y_row[:, bass.ts(g, group)],
                    in_=plg[:],
                    func=mybir.ActivationFunctionType.Sigmoid,
                    bias=tb2_sb[:],
                )

            nc.sync.dma_start(out=y_d[:, :], in_=y_row[:])

    nc.compile()
    return nc


_NC_CACHE = {}


def _get_nc():
    if "nc" not in _NC_CACHE:
        _NC_CACHE["nc"] = build_kernel()
    return _NC_CACHE["nc"]


TABLE_NP_DT = np.float16


def make_in_maps(dense_x, sparse_x, tables, w1, b1, w2, b2, tw1, tb1, tw2, tb2):
    tables_flat = np.ascontiguousarray(
        np.asarray(tables).reshape(F * CARD, D).astype(TABLE_NP_DT)
    )
    sparse_i32 = np.ascontiguousarray(np.asarray(sparse_x, dtype=np.int32))
    dense_f = np.ascontiguousarray(np.asarray(dense_x, dtype=np.float32))
    shared = {
        "tables": tables_flat,
        "w1": np.ascontiguousarray(np.asarray(w1, np.float32)),
        "b1": np.ascontiguousarray(np.asarray(b1, np.float32)),
        "w2": np.ascontiguousarray(np.asarray(w2, np.float32)),
        "b2": np.ascontiguousarray(np.asarray(b2, np.float32)),
        "tw1": np.ascontiguousarray(np.asarray(tw1, np.float32)),
        "tb1": np.ascontiguousarray(np.asarray(tb1, np.float32)),
        "tw2": np.ascontiguousarray(np.asarray(tw2, np.float32)),
        "tb2": np.ascontiguousarray(np.asarray(tb2, np.float32)),
    }
    b_loc = B // N_CORES
    in_maps = []
    for c in range(N_CORES):
        m = dict(shared)
        m["dense_x"] = dense_f[c * b_loc : (c + 1) * b_loc]
        m["sparse_x"] = sparse_i32[c * b_loc : (c + 1) * b_loc]
        in_maps.append(m)
    return in_maps


def kernel(**inputs):
    from concourse.bass_utils import run_bass_kernel_spmd

    nc = _get_nc()
    in_maps = make_in_maps(**inputs)
    res = run_bass_kernel_spmd(nc, in_maps, core_ids=list(range(N_CORES)))
    out = np.concatenate([r["y"].reshape(-1) for r in res.results])
    return out.reshape(B, 1).astype(np.float32)



# revision 2
# speedup vs baseline: 1.9144x; 1.9144x over previous
"""DLRM forward (embedding gather + tiny MLPs) as a Bass/Tile kernel on 8 trn2 cores.

Sharding: data-parallel over the batch. Each of the 8 cores gets B/8 = 2048 rows
of dense_x / sparse_x plus a full replica of the (read-only) embedding tables,
computes its batch shard end-to-end on device, and returns [1, 2048] sigmoid
outputs. The host only slices inputs and concatenates outputs.

v2 design (vs v1: fp16 tables + per-128-tile gathers + PE transposes):
  - tables quantized host-side to fp8e4 (x256 scale) -> 64B gather rows, half
    the HBM gather traffic; scale undone in the o1 activation (ALPHA).
  - flat gather indices (idx + f*CARD) precomputed on host, padded to 28
    rows/sample so each 128-sample frame is 1792B = 896 u16 (xbar-friendly).
  - one indirect DMA per 512-sample group (4 total) -> fewer SWDGE gens.
  - feature->partition transposes done by the DMA XBAR (dma_start
    transpose=True) on uint16 views: one instruction block-transposes
    [128 samples, 7x128 u16] -> [128 u16-feat, 7, 128 samples]; each u16
    carries an (even,odd) fp8 feature pair, which is exactly the DoubleRow
    matmul operand layout.
  - top-MLP layer 1 accumulated with fp8 DoubleRow matmuls (2 features per
    partition per pass), weights host-packed into [q, c, r, m] order.
  - dense_x host-transposed to [13, B] fp16 so the bottom MLP needs no PE
    transposes at all.
"""

import numpy as np
import ml_dtypes

import concourse.bass as bass
import concourse.mybir as mybir
import concourse.tile as tile
from concourse import bacc

P = 128

# Problem constants (hardcoded per harness contract).
N_CORES = 8
B = 16384
F = 26
D = 64
DENSE = 13
CARD = 100000
H_BOT = 8
H_TOP = 16

FPAD = 28                 # gathered rows per sample (26 + 2 pad -> 1792B frames)
FRAME = FPAD * D          # 1792 fp8 per sample-frame
KC = FRAME // 2 // P      # 7 u16 feature chunks of 128

S_T = 256.0               # host scale on tables before fp8e4 quantization
S_W = 64.0                # host scale on tw1 emb rows before fp8e4 quantization
ALPHA = 1.0 / (S_T * S_W)

f32 = mybir.dt.float32
i32 = mybir.dt.int32
fp16 = mybir.dt.float16
u16 = mybir.dt.uint16
f8 = mybir.dt.float8e4

USE_DOUBLE_ROW = True


def build_kernel(b_loc=B // N_CORES):
    group = 512
    n_g = b_loc // group
    tpg = group // P

    nc = bacc.Bacc("TRN2", target_bir_lowering=False)
    comb_d = nc.dram_tensor("comb", [P, n_g * tpg * FPAD], i32, kind="ExternalInput")
    tables_d = nc.dram_tensor("tables", [F * CARD, D], f8, kind="ExternalInput")
    dxt_d = nc.dram_tensor("dxt", [DENSE, b_loc], fp16, kind="ExternalInput")
    w1_d = nc.dram_tensor("w1", [DENSE, H_BOT], fp16, kind="ExternalInput")
    b1_d = nc.dram_tensor("b1", [H_BOT], f32, kind="ExternalInput")
    w2_d = nc.dram_tensor("w2", [H_BOT, D], fp16, kind="ExternalInput")
    b2_d = nc.dram_tensor("b2", [D], f32, kind="ExternalInput")
    tw1dr_d = nc.dram_tensor("tw1dr", [P, KC * 2 * H_TOP], f8, kind="ExternalInput")
    tw1d_d = nc.dram_tensor("tw1d", [D, H_TOP], fp16, kind="ExternalInput")
    tb1_d = nc.dram_tensor("tb1", [H_TOP], f32, kind="ExternalInput")
    tw2_d = nc.dram_tensor("tw2", [H_TOP, 1], fp16, kind="ExternalInput")
    tb2_d = nc.dram_tensor("tb2", [1], f32, kind="ExternalInput")
    y_d = nc.dram_tensor("y", [1, b_loc], f32, kind="ExternalOutput")

    with tile.TileContext(nc) as tc:
        with (
            tc.tile_pool(name="const", bufs=1) as cpool,
            tc.tile_pool(name="emb", bufs=2) as embp,
            tc.tile_pool(name="embT", bufs=2) as embtp,
            tc.tile_pool(name="small", bufs=2) as smallp,
            tc.tile_pool(name="po1", bufs=2, space="PSUM") as po1p,
            tc.tile_pool(name="psmall", bufs=2, space="PSUM") as psmallp,
        ):
            # ---- constants / inputs staged once ----
            comb_sb = cpool.tile([P, n_g * tpg * FPAD], i32)
            nc.sync.dma_start(out=comb_sb[:], in_=comb_d[:, :])
            dxt_sb = cpool.tile([DENSE, b_loc], fp16)
            nc.sync.dma_start(out=dxt_sb[:], in_=dxt_d[:, :])
            tw1dr_sb = cpool.tile([P, KC * 2 * H_TOP], f8)
            nc.sync.dma_start(out=tw1dr_sb[:], in_=tw1dr_d[:, :])
            tw1d_sb = cpool.tile([D, H_TOP], fp16)
            nc.sync.dma_start(out=tw1d_sb[:], in_=tw1d_d[:, :])
            tw2_sb = cpool.tile([H_TOP, 1], fp16)
            nc.sync.dma_start(out=tw2_sb[:], in_=tw2_d[:, :])
            w1_sb = cpool.tile([DENSE, H_BOT], fp16)
            nc.sync.dma_start(out=w1_sb[:], in_=w1_d[:, :])
            w2_sb = cpool.tile([H_BOT, D], fp16)
            nc.sync.dma_start(out=w2_sb[:], in_=w2_d[:, :])
            b1_sb = cpool.tile([H_BOT, 1], f32)
            nc.sync.dma_start(out=b1_sb[:], in_=b1_d[:, None])
            b2_sb = cpool.tile([D, 1], f32)
            nc.sync.dma_start(out=b2_sb[:], in_=b2_d[:, None])
            tb1_sb = cpool.tile([H_TOP, 1], f32)
            nc.sync.dma_start(out=tb1_sb[:], in_=tb1_d[:, None])
            tb2_sb = cpool.tile([1, 1], f32)
            nc.sync.dma_start(out=tb2_sb[:], in_=tb2_d[:, None])

            y_row = cpool.tile([1, b_loc], f32)

            for g in range(n_g):
                # ---- gather: one indirect DMA for the whole 512-sample group ----
                emb_g = embp.tile([P, tpg * FRAME], f8, tag="emb")
                nc.gpsimd.indirect_dma_start(
                    out=emb_g[:],
                    out_offset=None,
                    in_=tables_d[:, :],
                    in_offset=bass.IndirectOffsetOnAxis(
                        ap=comb_sb[:, bass.ts(g, tpg * FPAD)], axis=0
                    ),
                )

                # ---- feature->partition transpose via DMA XBAR on u16 views ----
                # in:  [128 samples, 896 u16]  (u16 = fp8 feature pair)
                # out: [128 u16-feat, 7 chunks, 128 samples]
                embt_g = embtp.tile([P, KC, group], u16, tag="embT")
                for j in range(tpg):
                    eng = nc.sync if j % 2 == 0 else nc.scalar
                    eng.dma_start(
                        out=embt_g[:, :, bass.ts(j, P)],
                        in_=emb_g[:, bass.ts(j, FRAME)].bitcast(u16),
                        transpose=True,
                    )

                # ---- bottom MLP (host-transposed dense, fp16) ----
                ph = psmallp.tile([H_BOT, group], f32, tag="psmall")
                nc.tensor.matmul(
                    out=ph[:], lhsT=w1_sb[:], rhs=dxt_sb[:, bass.ts(g, group)],
                    start=True, stop=True,
                )
                h_s = smallp.tile([H_BOT, group], fp16, tag="h")
                nc.scalar.activation(
                    out=h_s[:], in_=ph[:],
                    func=mybir.ActivationFunctionType.Relu, bias=b1_sb[:],
                )
                pd = psmallp.tile([D, group], f32, tag="psmall")
                nc.tensor.matmul(
                    out=pd[:], lhsT=w2_sb[:], rhs=h_s[:], start=True, stop=True
                )
                dt_sb = smallp.tile([D, group], fp16, tag="dt")
                nc.scalar.activation(
                    out=dt_sb[:], in_=pd[:],
                    func=mybir.ActivationFunctionType.Identity, bias=b2_sb[:],
                )

                # ---- top MLP layer 1: fp8 DoubleRow accumulation ----
                po1 = po1p.tile([H_TOP, group], f32, tag="po1")
                for c in range(KC):
                    lhs = tw1dr_sb[:, bass.ts(c, 2 * H_TOP)].rearrange(
                        "p (r m) -> p r m", r=2
                    )
                    rhs = embt_g[:, c, :].bitcast(f8).rearrange(
                        "p (s r) -> p r s", r=2
                    )
                    if USE_DOUBLE_ROW:
                        nc.tensor.matmul(
                            out=po1[:], lhsT=lhs, rhs=rhs,
                            start=(c == 0), stop=False,
                            perf_mode=mybir.MatmulPerfMode.DoubleRow,
                        )
                    else:
                        for r in range(2):
                            nc.tensor.matmul(
                                out=po1[:], lhsT=lhs[:, r, :], rhs=rhs[:, r, :],
                                start=(c == 0 and r == 0), stop=False,
                            )
                # dense chunk carries the same S_T*S_W scale (folded into tw1d)
                nc.tensor.matmul(
                    out=po1[:], lhsT=tw1d_sb[:], rhs=dt_sb[:],
                    start=False, stop=True,
                )

                o1 = smallp.tile([H_TOP, group], fp16, tag="o1")
                nc.scalar.activation(
                    out=o1[:], in_=po1[:],
                    func=mybir.ActivationFunctionType.Relu,
                    bias=tb1_sb[:], scale=ALPHA,
                )
                plg = psmallp.tile([1, group], f32, tag="psmall")
                nc.tensor.matmul(
                    out=plg[:], lhsT=tw2_sb[:], rhs=o1[:], start=True, stop=True
                )
                nc.scalar.activation(
                    out=y_row[:, bass.ts(g, group)], in_=plg[:],
                    func=mybir.ActivationFunctionType.Sigmoid, bias=tb2_sb[:],
                )

            nc.sync.dma_start(out=y_d[:, :], in_=y_row[:])

    nc.compile()
    return nc


_NC_CACHE = {}


def _get_nc():
    if "nc" not in _NC_CACHE:
        _NC_CACHE["nc"] = build_kernel()
    return _NC_CACHE["nc"]


def make_in_maps(dense_x, sparse_x, tables, w1, b1, w2, b2, tw1, tb1, tw2, tb2):
    b_loc = B // N_CORES
    group = 512
    n_g = b_loc // group
    tpg = group // P

    tables_q = np.ascontiguousarray(
        (np.asarray(tables, np.float32).reshape(F * CARD, D) * S_T).astype(
            ml_dtypes.float8_e4m3
        )
    )

    tw1_f = np.asarray(tw1, np.float32)
    tw1s = np.zeros((KC * 2 * P, H_TOP), np.float32)
    tw1s[: F * D] = tw1_f[: F * D] * S_W
    # [k, m] -> [c, q, r, m] -> [q, c, r, m]; k = c*256 + q*2 + r
    tw1dr = (
        tw1s.reshape(KC, P, 2, H_TOP)
        .transpose(1, 0, 2, 3)
        .reshape(P, KC * 2 * H_TOP)
        .astype(ml_dtypes.float8_e4m3)
    )
    tw1d = np.ascontiguousarray(
        (tw1_f[F * D : F * D + D] * (S_T * S_W)).astype(np.float16)
    )

    dense_f = np.asarray(dense_x, np.float32)
    sparse_i = np.asarray(sparse_x, np.int64).astype(np.int32)
    foffs = (np.arange(F, dtype=np.int32) * CARD)[None, :]

    shared = {
        "tables": tables_q,
        "tw1dr": np.ascontiguousarray(tw1dr),
        "tw1d": tw1d,
        "w1": np.ascontiguousarray(np.asarray(w1, np.float16)),
        "b1": np.ascontiguousarray(np.asarray(b1, np.float32)),
        "w2": np.ascontiguousarray(np.asarray(w2, np.float16)),
        "b2": np.ascontiguousarray(np.asarray(b2, np.float32)),
        "tb1": np.ascontiguousarray(np.asarray(tb1, np.float32)),
        "tw2": np.ascontiguousarray(np.asarray(tw2, np.float16)),
        "tb2": np.ascontiguousarray(np.asarray(tb2, np.float32)),
    }

    in_maps = []
    for cidx in range(N_CORES):
        sl = slice(cidx * b_loc, (cidx + 1) * b_loc)
        comb = np.zeros((b_loc, FPAD), np.int32)
        comb[:, :F] = sparse_i[sl] + foffs
        comb = np.ascontiguousarray(
            comb.reshape(n_g, tpg, P, FPAD)
            .transpose(2, 0, 1, 3)
            .reshape(P, n_g * tpg * FPAD)
        )
        dxt = np.ascontiguousarray(dense_f[sl].T.astype(np.float16))
        m = dict(shared)
        m["comb"] = comb
        m["dxt"] = dxt
        in_maps.append(m)
    return in_maps


def kernel(**inputs):
    from concourse.bass_utils import run_bass_kernel_spmd

    nc = _get_nc()
    in_maps = make_in_maps(**inputs)
    res = run_bass_kernel_spmd(nc, in_maps, core_ids=list(range(N_CORES)))
    out = np.concatenate([r["y"].reshape(-1) for r in res.results])
    return out.reshape(B, 1).astype(np.float32)
